# revision 1
# baseline (speedup 1.0000x reference)
# Trainium2 Bass kernel for nn_Action2 (invest-consumption SDE scan with two
# small MLPs per step). Data-parallel across 8 NeuronCores: batch 8192 -> 1024
# per core, split into TWO independent 512-wide streams whose serial
# update chains interleave (one stream's matmuls fill the other's
# elementwise-update latency). Host prep (BatchNorm stats, drift terms,
# bf16 hi/lo splits, layout shuffles) is plain numpy.
import os
import sys

import numpy as np

for _p in ("/opt/trn_rl_repo",):
    if _p not in sys.path:
        sys.path.insert(0, _p)

import ml_dtypes  # noqa: E402

import concourse.bacc as bacc  # noqa: E402
import concourse.mybir as mybir  # noqa: E402
import concourse.tile as tile  # noqa: E402
from concourse.bass_utils import run_bass_kernel_spmd  # noqa: E402
from concourse.tile_rust import add_dep_helper  # noqa: E402

F32 = mybir.dt.float32
F16 = mybir.dt.float16
BF16 = mybir.dt.bfloat16
BF16_NP = ml_dtypes.bfloat16
ALU = mybir.AluOpType
ACTF = mybir.ActivationFunctionType

B_GLOBAL = 8192
N_CORES = 8
B = B_GLOBAL // N_CORES   # 1024 per core
SB = B // 2               # 512 per stream
N_STEPS = 100
IN_DIM = 5
T_HORIZON = 1.0
MU, NU, SIGMA = 0.1, 0.2, 0.3
BN_EPS = 1e-5
DT = T_HORIZON / N_STEPS

# Per-stream local batch index: bs = 256*cl + 32*fb + q  (bs in [0,512))
#   cl in [0,2): 256-column chunk -> update-domain partition block 32*cl
#   fb in [0,8), q in [0,32): update partition = 32*cl + q (64 partitions)
# Global local index b = 512*s + bs. 128-col chunk j_loc = 2*cl + c2,
# c2 = fb // 4, m = fb % 4. Xhl (64,128) col = 32*m + 4*c2 + t.

VE_RELU = 256  # relu columns on VectorE per stream (rest on ScalarE)


def _split_bf16(x):
    x = np.asarray(x, np.float32)
    h = x.astype(BF16_NP).astype(np.float32)
    l = (x - h).astype(BF16_NP).astype(np.float32)
    return h, l


def _feature_rows(f):
    fh, fl = _split_bf16(f)
    return [fh, fl, fh, fl]


def _weight_rows(w):
    wh, wl = _split_bf16(w)
    return [wh, wh, wl, wl]


def build(n_steps=N_STEPS):
    nc = bacc.Bacc("TRN2", target_bir_lowering=False, debug=False)

    def din(name, shape, dtype):
        return nc.dram_tensor(name, list(shape), dtype, kind="ExternalInput").ap()

    STATIC40 = din("static40", (40, B), BF16)
    MXMC = din("mxmc", (n_steps, 8, B), BF16)
    D_ALL = din("d_all", (2, 64, 8 * n_steps), F32)
    B1EFF = din("b1eff", (128, n_steps), F32)
    B2EFF = din("b2eff", (128, 1), F32)
    W1A_D = din("w1a", (48, 128), BF16)
    W1XJ_D = din("w1xj", (2, 4, 64, 128), F16)   # per-stream zero-padded x lhsTs
    XROWS0 = din("xrows0", (64, 128), F16)
    W2S_D = din("w2s", (128, 128), F32)
    W3S_D = din("w3s", (128, 2), F32)
    SCAL = din("scal", (64, 2), F32)

    OUT = nc.dram_tensor("out", [128, 8 * (n_steps + 1)], F32, kind="ExternalOutput").ap()

    with tile.TileContext(nc) as tc:
        import contextlib

        with contextlib.ExitStack() as ctx:
            const = ctx.enter_context(tc.tile_pool(name="const", bufs=1))
            stgp = ctx.enter_context(tc.tile_pool(name="stg", bufs=1))
            h1p = ctx.enter_context(tc.tile_pool(name="h1", bufs=2))
            h2p = ctx.enter_context(tc.tile_pool(name="h2", bufs=2))
            updp = ctx.enter_context(tc.tile_pool(name="upd", bufs=2))
            ps1 = ctx.enter_context(tc.tile_pool(name="ps1", bufs=2, space="PSUM"))
            ps2 = ctx.enter_context(tc.tile_pool(name="ps2", bufs=1, space="PSUM"))
            ps3 = ctx.enter_context(tc.tile_pool(name="ps3", bufs=1, space="PSUM"))

            w1a = const.tile([48, 128], BF16)
            nc.sync.dma_start(w1a[:], W1A_D)
            w1xj = [const.tile([64, 4 * 128], F16, tag=f"w1xj{s}", name=f"w1xj{s}")
                    for s in range(2)]
            for s in range(2):
                for j in range(4):
                    nc.sync.dma_start(w1xj[s][:, 128 * j:128 * (j + 1)], W1XJ_D[s, j])
            w2f = const.tile([128, 128], F32)
            nc.sync.dma_start(w2f[:], W2S_D)
            w2r = const.tile([128, 128], F16)
            nc.vector.tensor_copy(w2r[:], w2f[:])
            w3f = const.tile([128, 2], F32)
            nc.sync.dma_start(w3f[:], W3S_D)
            w3b = const.tile([128, 2], F16)
            nc.vector.tensor_copy(w3b[:], w3f[:])
            b1eff = const.tile([128, n_steps], F32)
            nc.sync.dma_start(b1eff[:], B1EFF)
            b2eff = const.tile([128, 1], F32)
            nc.sync.dma_start(b2eff[:], B2EFF)
            d_all = [const.tile([64, 8 * n_steps], F32, tag=f"dall{s}", name=f"dall{s}")
                     for s in range(2)]
            for s in range(2):
                nc.sync.dma_start(d_all[s][:], D_ALL[s])
            scal = const.tile([64, 2], F32)
            nc.sync.dma_start(scal[:], SCAL)
            b3s = scal[:, 0:1]
            bc3s = scal[:, 1:2]

            xh_hist = [const.tile([64, 8 * (n_steps + 1)], F32, tag=f"xh{s}", name=f"xh{s}")
                       for s in range(2)]
            for s in range(2):
                nc.vector.memset(xh_hist[s][:], 1.0)

            p3sp = [ps3.tile([64, 256], F32, tag=f"p3sp{s}", name=f"p3sp{s}")
                    for s in range(2)]
            for s in range(2):
                nc.vector.memset(p3sp[s][:], 0.0)

            xrows = [[const.tile([64, 128], F16, tag=f"xr{s}{k}", name=f"xr{s}{k}")
                      for k in range(2)] for s in range(2)]
            for s in range(2):
                nc.sync.dma_start(xrows[s][0][:], XROWS0)
            xhl = [[const.tile([64, 128], F16, tag=f"xhl{s}{k}", name=f"xhl{s}{k}")
                    for k in range(2)] for s in range(2)]
            for s in range(2):
                for k in range(2):
                    nc.vector.memset(xhl[s][k][:], 0.0)

            stg = [stgp.tile([48, B], BF16, tag=f"stgt{k}", name=f"stgt{k}")
                   for k in range(3)]
            for k in range(3):
                nc.sync.dma_start(stg[k][0:40, :], STATIC40)

            def mm1a(i, s):
                st = stg[i % 3]
                p1 = ps1.tile([128, SB], F32, tag=f"p1{s}", name=f"p1_{s}_{i}")
                nc.tensor.matmul(p1[:], w1a[:], st[:, 512 * s:512 * (s + 1)],
                                 start=True, stop=False)
                return p1

            nc.sync.dma_start(stg[0][40:48, :], MXMC[0])
            if n_steps > 1:
                nc.sync.dma_start(stg[1][40:48, :], MXMC[1])
            p1_next = [mm1a(0, 0), mm1a(0, 1)]
            skew_anchor = None

            for i in range(n_steps):
                if i + 2 < n_steps:
                    nc.sync.dma_start(stg[(i + 2) % 3][40:48, :], MXMC[i + 2])
                for s in range(2):
                    xr = xrows[s][i % 2]
                    p1 = p1_next[s]

                    # L1 x-term: 4 chunk matmuls, zero-padded K=64 lhsTs
                    w1xj_r = w1xj[s][:].rearrange("p (j n) -> p j n", j=4)
                    for j in range(4):
                        mm_x = nc.tensor.matmul(
                            p1[:, 128 * j:128 * (j + 1)], w1xj_r[:, j, :],
                            xr[:], start=False, stop=(j == 3))
                        if i == 0 and s == 1 and j == 0 and skew_anchor is not None:
                            # half-step skew: push the streams into anti-phase
                            add_dep_helper(mm_x.ins, skew_anchor, sync=True,
                                           reason="stream anti-phase skew")

                    h1 = h1p.tile([128, SB], F16, tag=f"h1{s}", name=f"h1_{s}_{i}")
                    bcol = b1eff[:, i:i + 1]
                    nc.vector.tensor_scalar(
                        h1[:, 0:VE_RELU], p1[:, 0:VE_RELU], bcol, 0.0,
                        ALU.add, ALU.max)
                    nc.scalar.activation(
                        h1[:, VE_RELU:SB], p1[:, VE_RELU:SB], ACTF.Relu,
                        bias=bcol)

                    # L2
                    p2 = ps2.tile([128, SB], F32, tag=f"p2{s}", name=f"p2_{s}_{i}")
                    nc.tensor.matmul(p2[:], w2r[:], h1[:], start=True, stop=True)
                    h2 = h2p.tile([128, SB], F16, tag=f"h2{s}", name=f"h2_{s}_{i}")
                    nc.vector.tensor_scalar(
                        h2[:, 0:VE_RELU], p2[:, 0:VE_RELU], b2eff[:], 0.0,
                        ALU.add, ALU.max)
                    nc.scalar.activation(
                        h2[:, VE_RELU:SB], p2[:, VE_RELU:SB], ACTF.Relu,
                        bias=b2eff[:])

                    # L3: 2 chunk matmuls on column groups 0 / 32
                    for cl in range(2):
                        mm3 = nc.tensor.matmul(
                            p3sp[s][32 * cl:32 * cl + 2, :], w3b[:],
                            h2[:, 256 * cl:256 * (cl + 1)], start=True, stop=True,
                            tile_position=(0, 32 * cl))
                        if i == 0 and s == 0 and cl == 1:
                            skew_anchor = mm3.ins

                    if s == 1 and i + 1 < n_steps:
                        p1_next = [mm1a(i + 1, 0), mm1a(i + 1, 1)]

                    # crossing to batch space (64-partition update domain)
                    p3t = updp.tile([64, 256], F32, tag=f"p3t{s}", name=f"p3t_{s}_{i}")
                    nc.vector.transpose(p3t[:], p3sp[s][:])
                    p3t_r = p3t[:].rearrange("p (f t) -> p f t", t=32)
                    pi_view = p3t_r[:, :, 0]
                    lc_view = p3t_r[:, :, 1]

                    cdt = updp.tile([64, 8], F32, tag=f"cdt{s}", name=f"cdt_{s}_{i}")
                    nc.scalar.activation(cdt[:], lc_view, ACTF.Exp, bias=bc3s)
                    a_t = updp.tile([64, 8], F32, tag=f"a{s}", name=f"a_{s}_{i}")
                    nc.vector.scalar_tensor_tensor(
                        a_t[:], pi_view, b3s, d_all[s][:, 8 * i:8 * (i + 1)],
                        ALU.add, ALU.mult)
                    g_t = updp.tile([64, 8], F32, tag=f"g{s}", name=f"g_{s}_{i}")
                    nc.vector.scalar_tensor_tensor(
                        g_t[:], a_t[:], 1.0, cdt[:], ALU.add, ALU.subtract)

                    xprev = xh_hist[s][:, 8 * i:8 * (i + 1)]
                    xnext = xh_hist[s][:, 8 * (i + 1):8 * (i + 2)]
                    nc.vector.tensor_mul(xnext, xprev, g_t[:])

                    # rebuild x feature rows (single fp16 row per chunk)
                    xhl_t = xhl[s][i % 2]
                    dst = xhl_t[:].rearrange("p (m q2 t) -> p m q2 t", m=4, q2=8, t=4)
                    src = xnext.rearrange("p (c2 m) -> p c2 m", c2=2, m=4)
                    src = src.transpose([0, 2, 1])
                    nc.vector.tensor_copy(dst[:, :, 0:2, 0], src)
                    nc.vector.transpose(xrows[s][(i + 1) % 2][:], xhl_t[:])

            for s in range(2):
                nc.sync.dma_start(OUT[64 * s:64 * (s + 1), :], xh_hist[s][:])

    nc.compile()
    return nc


def host_prep(inputs, n_steps=N_STEPS):
    bm = np.asarray(inputs["bm"], np.float32)
    cn = np.asarray(inputs["cn"], np.float32)
    typeVec = np.asarray(inputs["typeVec"], np.float32)
    mx = np.asarray(inputs["mx"], np.float32)
    mc = np.asarray(inputs["mc"], np.float32)
    initial = float(np.asarray(inputs["initial"]).reshape(-1)[0])
    bn_gamma = np.asarray(inputs["bn_gamma"], np.float32)
    bn_beta = np.asarray(inputs["bn_beta"], np.float32)
    bnc_gamma = np.asarray(inputs["bnc_gamma"], np.float32)
    bnc_beta = np.asarray(inputs["bnc_beta"], np.float32)
    w1 = np.asarray(inputs["w1"], np.float32)
    b1 = np.asarray(inputs["b1"], np.float32)
    w2 = np.asarray(inputs["w2"], np.float32)
    b2 = np.asarray(inputs["b2"], np.float32)
    w3 = np.asarray(inputs["w3"], np.float32)
    b3 = np.asarray(inputs["b3"], np.float32)
    wc1 = np.asarray(inputs["wc1"], np.float32)
    bc1 = np.asarray(inputs["bc1"], np.float32)
    wc2 = np.asarray(inputs["wc2"], np.float32)
    bc2 = np.asarray(inputs["bc2"], np.float32)
    wc3 = np.asarray(inputs["wc3"], np.float32)
    bc3 = np.asarray(inputs["bc3"], np.float32)

    Bg, N, _ = bm.shape
    assert Bg == B_GLOBAL and N >= n_steps

    m = typeVec.mean(axis=0, dtype=np.float64)
    v = ((typeVec.astype(np.float64) - m) ** 2).mean(axis=0)
    inv = 1.0 / np.sqrt(v + BN_EPS)
    bn = ((typeVec - m) * inv * bn_gamma + bn_beta).astype(np.float32)
    bnc = ((typeVec - m) * inv * bnc_gamma + bnc_beta).astype(np.float32)

    dcn = cn[:, 1:n_steps + 1, 0] - cn[:, :n_steps, 0]
    drift = (np.float32(MU * DT) + np.float32(NU) * bm[:, :n_steps, 0]
             + np.float32(SIGMA) * dcn).astype(np.float32)
    mxs = mx[:, :n_steps, 0]
    mcs = mc[:, :n_steps, 0]
    ts = (np.arange(n_steps, dtype=np.float32) * np.float32(DT))

    def stack_row(a, b_):
        return np.concatenate([a, b_]).astype(np.float32)

    w1a = np.zeros((48, 128), np.float32)
    for k in range(5):
        wpi = np.zeros(128, np.float32)
        wpi[0:64] = w1[k]
        w1a[4 * k:4 * k + 4] = np.stack(_weight_rows(wpi))
    for k in range(5):
        wcc = np.zeros(128, np.float32)
        wcc[64:128] = wc1[k]
        w1a[20 + 4 * k:24 + 4 * k] = np.stack(_weight_rows(wcc))
    w1a[40:44] = np.stack(_weight_rows(stack_row(w1[7], wc1[7])))
    w1a[44:48] = np.stack(_weight_rows(stack_row(w1[8], wc1[8])))

    wx = stack_row(w1[6], wc1[6])
    w1xj = np.zeros((2, 4, 64, 128), np.float32)
    for s in range(2):
        for j in range(4):
            cl, c2 = j // 2, j % 2
            w1xj[s, j, 32 * cl + 4 * c2] = wx

    b1eff = (np.concatenate([b1, bc1])[None, :]
             + ts[:, None] * stack_row(w1[5], wc1[5])[None, :]).astype(np.float32).T
    b2eff = np.concatenate([b2, bc2]).astype(np.float32).reshape(128, 1)
    w2s = np.zeros((128, 128), np.float32)
    w2s[0:64, 0:64] = w2
    w2s[64:128, 64:128] = wc2
    w3s = np.zeros((128, 2), np.float32)
    w3s[0:64, 0] = w3[:, 0]
    w3s[64:128, 1] = wc3[:, 0]
    scal = np.zeros((64, 2), np.float32)
    scal[:, 0] = b3[0]
    scal[:, 1] = np.float32(bc3[0] + np.log(DT))

    # per-stream index maps: bs = 256*cl + 32*fb + q, update partition 32*cl+q
    bs = np.arange(SB)
    cl_i, fb_i, q_i = bs // 256, (bs % 256) // 32, bs % 32
    p_i = 32 * cl_i + q_i                      # (512,) per-stream partition

    xrows0 = np.zeros((64, 128), np.float32)
    for j in range(4):
        xrows0[32 * (j // 2) + 4 * (j % 2)] = initial

    in_maps = []
    for core in range(N_CORES):
        sl = slice(core * B, (core + 1) * B)
        bn_c, bnc_c = bn[sl], bnc[sl]
        static40 = np.zeros((40, B), np.float32)
        for k in range(5):
            static40[4 * k:4 * k + 4] = np.stack(_feature_rows(bn_c[:, k]))
        for k in range(5):
            static40[20 + 4 * k:24 + 4 * k] = np.stack(_feature_rows(bnc_c[:, k]))
        mxmc = np.zeros((n_steps, 8, B), np.float32)
        mxc, mcc = mxs[sl], mcs[sl]
        for i in range(n_steps):
            mxmc[i, 0:4] = np.stack(_feature_rows(mxc[:, i]))
            mxmc[i, 4:8] = np.stack(_feature_rows(mcc[:, i]))
        d_np = np.zeros((2, 64, 8 * n_steps), np.float32)
        dr = drift[sl]
        for s in range(2):
            drs = dr[512 * s:512 * (s + 1)]    # (512, n_steps)
            d_np[s][p_i[:, None], 8 * np.arange(n_steps)[None, :] + fb_i[:, None]] = drs
        in_maps.append({
            "static40": static40.astype(BF16_NP),
            "mxmc": mxmc.astype(BF16_NP),
            "d_all": d_np,
            "b1eff": b1eff.copy(),
            "b2eff": b2eff.copy(),
            "w1a": w1a.astype(BF16_NP),
            "w1xj": w1xj.astype(np.float16),
            "xrows0": xrows0.astype(np.float16),
            "w2s": w2s.copy(),
            "w3s": w3s.copy(),
            "scal": scal.copy(),
        })
    decode = (p_i, fb_i)
    return in_maps, decode


def assemble_output(results, decode, n_steps=N_STEPS):
    p_i, fb_i = decode
    states = np.empty((B_GLOBAL, n_steps + 1), np.float32)
    cols = 8 * np.arange(n_steps + 1)[None, :] + fb_i[:, None]
    for core in range(N_CORES):
        out = results[core]["out"]              # (128, 8*(n_steps+1))
        for s in range(2):
            rows = 64 * s + p_i
            states[core * B + 512 * s:core * B + 512 * (s + 1)] = out[rows[:, None], cols]
    times = (np.arange(n_steps + 1, dtype=np.float32) * np.float32(DT))
    full = np.empty((B_GLOBAL, n_steps + 1, 2), np.float32)
    full[:, :, 0] = times[None, :]
    full[:, :, 1] = states
    return full


_BUILT = {}


def _get_built(n_steps=N_STEPS):
    if n_steps not in _BUILT:
        _BUILT[n_steps] = build(n_steps)
    return _BUILT[n_steps]


def kernel(**inputs):
    nc = _get_built()
    in_maps, decode = host_prep(inputs)
    res = run_bass_kernel_spmd(nc, in_maps, core_ids=list(range(N_CORES)))
    return assemble_output(res.results, decode)


if __name__ == "__main__":
    sys.path.insert(0, os.path.dirname(os.path.abspath(__file__)))
    import reference

    inputs = reference.setup_inputs()
    inputs = {k: np.asarray(v) for k, v in inputs.items()}
    expected = np.asarray(reference.reference(**inputs))
    actual = kernel(**inputs)
    err = np.abs(actual - expected)
    print("max abs err:", err.max())
    print("rel err (scale):", err.max() / np.abs(expected).max())

